# revision 13
# baseline (speedup 1.0000x reference)
"""TRN2 Bass kernel for nn_BalancedHamiltonLayer.

Math: out[n,k,j] = sum_{r,s,i} x[n,s,i] * factors_B[r,j,i] * H(A)[r,k,s] + bias
collapses to a single dense matmul  out = x2d @ W + bias  with
W[(s,i),(k,j)] = sum_r H[r,k,s] * B[r,j,i]  (a 1024x1024 matrix folded on host
in float64).

Sharding: data-parallel over the 8192 token rows across 8 NeuronCores
(1024 rows each); W replicated.  The matmul runs in fp16 on the PE
(full-rate, FWL weight loads, ~3e-4 relative error; fp32 PSUM
accumulation).

Schedule notes (from perfetto traces):
- The NEFF preamble blocks each sequencer until ~6-7us; Scalar's HWDGE
  ring frees ~0.9us before Sync's, so ALL loads issue on nc.scalar in
  strict deadline order (single FIFO ring -> early chunks complete
  early), and all stores go on the otherwise-idle nc.sync ring.
- x is staged per k-chunk as xk[k] = [128 partitions, 1024 tokens]
  (host pre-transposed), so every load is 2KB-contiguous per partition
  and phase 1 can start after just xk0 + half of w0.
- PE HAM warm-up: ~3.4us of N=128 matmuls on a DVE-zeroed tile starting
  right when the Tensor queue opens, so the 2.4 GHz clock gate is open
  when the real matmuls start.  The warm PSUM buffer is recycled by the
  pool as pts[3][1] (WAW-ordered).
- Output is stored as fp16 (host upcasts and adds bias); the last
  m-tile is computed in four N=256 quarters so only a 64KB store
  remains after the final matmul.
"""

import numpy as np
import concourse.bacc as bacc
import concourse.mybir as mybir
import concourse.tile as tile
from concourse.bass_utils import run_bass_kernel_spmd

B, T, D = 4, 2048, 1024
RANK, FACTOR, SUB = 8, 64, 4
S = 4 * SUB  # 16
NCORES = 8
NTOK = B * T // NCORES  # 1024 token rows per core
P = 128
KT = D // P     # 8 contraction chunks
MT = NTOK // P  # 8 token tiles per core
NH = 512        # f_out half (one PSUM bank)
NWARM = 32      # N=128 warm-up matmuls (~3.4us) -- bridges the head-DMA
                # completion-semaphore latency (~1.2us past data arrival,
                # one 16-inc sem chain per DMA, FIFO per ring) so the PE
                # never idles >MID-window and every real matmul runs warm

_cached_nc = None


def build_module():
    global _cached_nc
    if _cached_nc is not None:
        return _cached_nc
    nc = bacc.Bacc("TRN2", target_bir_lowering=False, debug=False)
    xT = nc.dram_tensor("xT", [KT, P, NTOK], mybir.dt.float16, kind="ExternalInput").ap()
    w = nc.dram_tensor("w", [D, D], mybir.dt.float16, kind="ExternalInput").ap()
    out = nc.dram_tensor("out", [NTOK, D], mybir.dt.float16, kind="ExternalOutput").ap()

    with tile.TileContext(nc) as tc:
        with (
            tc.tile_pool(name="wp", bufs=1) as wp,
            tc.tile_pool(name="xp", bufs=1) as xp,
            tc.tile_pool(name="op", bufs=4) as op,
            tc.tile_pool(name="ps", bufs=4, space="PSUM") as ps,
        ):
            # Zero tile for warm-up matmuls: DVE memset (Vector's queue
            # opens first after the preamble; no GpSimd involvement).
            z = xp.tile([P, P], mybir.dt.float16, tag="warm", name="z")
            nc.vector.memset(z[:], 0.0)

            # Warm PSUM buffer: first instance of tag ps1 -- the pool
            # hands this same bank back out as pts[3][1] later, with a
            # WAW dependency on the warm matmuls (same engine, in-order).
            warm = ps.tile([P, NH], mybir.dt.float32, tag="ps1", name="warm")

            # One tile = one DMA = one 16-inc completion-sem chain.  The
            # sem chains drain FIFO per ring at ~(data + 1.2us), and the
            # Tile scheduler coalesces matmul waits across neighbouring
            # DMAs -- so FEWER, LARGER loads reach sem-visibility sooner
            # than clever small splits (measured: splits cost 1-2us).
            xt = {}
            wt = {}
            for k in range(KT):
                xt[k] = xp.tile([P, NTOK], mybir.dt.float16, tag=f"x{k}", name=f"xt{k}")
                wt[k] = wp.tile([P, 2 * NH], mybir.dt.float16, tag=f"w{k}", name=f"wt{k}")

            def lhs_of(k, m):
                return xt[k][:, m * P:(m + 1) * P]

            def rhs_of(k, n):
                return wt[k][:, n * NH:(n + 1) * NH]

            # w0 alone on the sync ring: its sem chain runs in parallel
            # with xk0's on scalar, so the first matmul's deps are two
            # concurrent 256KB transfers.  Everything else streams on
            # scalar in strict deadline order; sync then serves stores.
            nc.sync.dma_start(wt[0][:], w[0:P, :])
            loads = [
                (xt[0][:], xT[0]),
                (xt[1][:], xT[1]),
                (wt[1][:], w[P:2 * P, :]),
            ]
            for k in range(2, KT):
                loads.append((xt[k][:], xT[k]))
                loads.append((wt[k][:], w[k * P:(k + 1) * P, :]))
            for da, sa in loads:
                nc.scalar.dma_start(da, sa)

            store_ring = [0]

            def emit_out(m, n0, c0, width, pt):
                o = op.tile([P, width], mybir.dt.float16, tag="o", name="o")
                nc.vector.tensor_copy(o[:], pt)
                eng = nc.sync if store_ring[0] % 2 == 0 else nc.scalar
                store_ring[0] += 1
                eng.dma_start(out[m * P:(m + 1) * P, c0:c0 + width], o[:])

            def emit_row(m, pt0, pt1):
                # Full-row store: both PSUM halves copied into one fp16
                # tile, one DMA -- fewer completion-sem chains to sweep at
                # context end.
                o = op.tile([P, 2 * NH], mybir.dt.float16, tag="o", name="o")
                nc.vector.tensor_copy(o[:, 0:NH], pt0)
                nc.vector.tensor_copy(o[:, NH:2 * NH], pt1)
                eng = nc.sync if store_ring[0] % 2 == 0 else nc.scalar
                store_ring[0] += 1
                eng.dma_start(out[m * P:(m + 1) * P, :], o[:])

            with nc.named_scope("mm"):
                for i in range(NWARM):
                    nc.tensor.matmul(
                        warm[:, 0:P], z[:], z[:], start=True, stop=True
                    )

                # Phase 1: m=0..3 k-interleaved across all 8 PSUM banks --
                # per-k compute (8 MMs, ~1.76us) exceeds the load arrival
                # cadence (~1.5us per k), so the PE absorbs DMA jitter.
                NP1 = 4
                pts = {
                    m: {
                        n: ps.tile([P, NH], mybir.dt.float32, tag=f"ps{n}", name=f"pt{m}_{n}")
                        for n in range(2)
                    }
                    for m in range(NP1)
                }
                for k in range(KT):
                    for m in range(NP1):
                        for n in range(2):
                            nc.tensor.matmul(
                                pts[m][n][:],
                                lhs_of(k, m),
                                rhs_of(k, n),
                                start=(k == 0),
                                stop=(k == KT - 1),
                            )
                for m in range(NP1):
                    emit_row(m, pts[m][0][:], pts[m][1][:])

                # Phase 2: m=4..6 k-contiguous per m-tile (PE stays warm).
                for m in range(NP1, MT - 1):
                    pt = {
                        n: ps.tile([P, NH], mybir.dt.float32, tag=f"ps{n}", name=f"pt{n}")
                        for n in range(2)
                    }
                    for k in range(KT):
                        for n in range(2):
                            nc.tensor.matmul(
                                pt[n][:],
                                lhs_of(k, m),
                                rhs_of(k, n),
                                start=(k == 0),
                                stop=(k == KT - 1),
                            )
                    emit_row(m, pt[0][:], pt[1][:])

                # Last m-tile in four N=256 quarters, staggered so the
                # copy+store of quarters 0-2 overlap quarter 3's matmuls.
                m = MT - 1
                NQ = 256

                def rhs_q(k, q):
                    return wt[k][:, q * NQ:(q + 1) * NQ]

                for q in range(4):
                    pq = ps.tile([P, NQ], mybir.dt.float32, tag=f"ps{q % 2}", name=f"pq{q}")
                    for k in range(KT):
                        nc.tensor.matmul(
                            pq[:],
                            lhs_of(k, m),
                            rhs_q(k, q),
                            start=(k == 0),
                            stop=(k == KT - 1),
                        )
                    if q == 3:
                        # same-engine copy+store: ACT copies PSUM->SBUF,
                        # then its own HWDGE ring stores -- no cross-
                        # engine sem hop on the kernel's tail.
                        o = op.tile([P, NQ], mybir.dt.float16, tag="o", name="o")
                        nc.scalar.copy(o[:], pq[:])
                        nc.scalar.dma_start(
                            out[m * P:(m + 1) * P, q * NQ:(q + 1) * NQ], o[:]
                        )
                    else:
                        emit_out(m, q // 2, q * NQ, NQ, pq[:])
    nc.compile()
    _cached_nc = nc
    return nc


def _construct_hamilton(A):
    # A: [rank, 4, sub, sub] -> [rank, 4*sub, 4*sub]
    r, i, j, k = A[:, 0], A[:, 1], A[:, 2], A[:, 3]
    return np.concatenate(
        [
            np.concatenate([r, -i, -j, -k], axis=2),
            np.concatenate([i, r, -k, j], axis=2),
            np.concatenate([j, k, r, -i], axis=2),
            np.concatenate([k, -j, i, r], axis=2),
        ],
        axis=1,
    )


def build_in_maps(x, A, factors_B):
    H = _construct_hamilton(np.asarray(A, dtype=np.float64))  # [r, k, s]
    Bf = np.asarray(factors_B, dtype=np.float64)  # [r, j, i]
    # W[(s,i),(k,j)] = sum_r H[r,k,s] * B[r,j,i]
    W = np.einsum("rks,rji->sikj", H, Bf).reshape(D, D).astype(np.float16)

    x2 = np.asarray(x, dtype=np.float16).reshape(NCORES, NTOK, D)
    in_maps = []
    for c in range(NCORES):
        # [NTOK, D] -> [D, NTOK] -> [KT, P, NTOK]
        xs = np.ascontiguousarray(x2[c].T).reshape(KT, P, NTOK)
        in_maps.append({"xT": xs, "w": W})
    return in_maps


def kernel(x, A, factors_B, bias):
    nc = build_module()
    in_maps = build_in_maps(x, A, factors_B)
    br = run_bass_kernel_spmd(nc, in_maps, core_ids=list(range(NCORES)))
    out = np.concatenate([r["out"].astype(np.float32) for r in br.results], axis=0)
    out = out + np.asarray(bias, dtype=np.float32)[None, :]
    return out.reshape(B, T, D)
